# revision 1
# baseline (speedup 1.0000x reference)
"""GAT layer kernel for 8 Trainium2 NeuronCores.

Row-shards the N=8192 destination nodes across 8 cores (1024 rows each).
Each core receives its adjacency slice pre-transposed (adjT[j, i] =
adj[row_i, j], same bytes, sharding layout choice), so all compute runs in
the matmul-ready [j, i] layout and no on-device transpose is needed.

Per core, 8 "stacks", each stack s covering 8 j-blocks of 128 (j in
[s*1024, (s+1)*1024)), as one [128, 8*1024] fp16 tile:
  adjT --dma+cast--> adjf16 (0.0/1.0)                       (SWDGE, int32->fp16)
  per j-block b: z[:,b,:] = leaky_relu(f_i + g_j, 0.2)
      ACT path:  Prelu(in=F, bias=g_b, alpha=0.2)           (1 op/block)
      DVE path:  ts add (f + g_b), then stacked in-place
                 scalar_tensor_tensor max(0.2*z, z)
  z = exp(z - 2)           stacked, ACT (the -2 cancels in softmax; keeps fp16)
  z = z * adjf16           stacked, DVE (mask -> exact zeros)
  per block b, i-chunk c:  psum_c[65, 512] += haug[b].T @ z  (PE; haug=[h|1],
                           row 64 accumulates the softmax denominator)
Epilogue: PE-transpose psum 128-col slices, divide rows by the denominator,
Prelu(0.01), DMA out (natural [1024, 64] layout).

h = input@W, f = h@a[:64], g = h@a[64:] are precomputed on host (the
"replicate h" strategy from the sharding hint - O(N*F) vs O(N^2) on device).
"""

import sys

if "/opt/trn_rl_repo" not in sys.path:
    sys.path.insert(0, "/opt/trn_rl_repo")

import numpy as np

N = 8192
F_OUT = 64
NCORES = 8
ROWS = N // NCORES  # 1024 rows per core
P = 128
JB = N // P         # 64 j-blocks
NS = 8              # stacks per core
KB = JB // NS       # 8 j-blocks per stack
HA_W = F_OUT + 1    # h features + ones column
CS = 2.0            # exp shift (cancels in softmax)
ACT_STACKS = (1, 3, 6)  # stacks whose leaky-relu runs on ACT (Prelu)
POOL_MASKS = ()         # gpsimd mask offload (off: proven regression on HW)
ACT_BLOCKS = {4: 4}     # stack 4: first 4 blocks on ACT (a=3.5, HW-proven -12us)

_nc_cache = {}


def build_bass(act_stacks=ACT_STACKS, reps=1, sim_relu=False, pool_masks=POOL_MASKS,
               ns=NS, bufs=4, act_blocks=ACT_BLOCKS):
    from contextlib import ExitStack

    import concourse.bacc as bacc
    import concourse.tile as tile
    from concourse import mybir
    from concourse.masks import make_identity

    f16 = mybir.dt.float16
    f32 = mybir.dt.float32
    i32 = mybir.dt.int32
    Alu = mybir.AluOpType
    Act = mybir.ActivationFunctionType

    # sim_relu: the interp lacks Prelu; Relu has identical cost (timing-only runs)
    PRELU = Act.Relu if sim_relu else Act.Prelu

    nc = bacc.Bacc()
    adj_d = nc.declare_dram_parameter("adjt", [N, ROWS], i32, isOutput=False)
    fb_d = nc.declare_dram_parameter("fbig", [P, ROWS], f16, isOutput=False)
    g_d = nc.declare_dram_parameter("gsc", [P, JB], f32, isOutput=False)
    ha_d = nc.declare_dram_parameter("haug", [P, JB, HA_W], f16, isOutput=False)
    out_d = nc.declare_dram_parameter("out", [ROWS, F_OUT], f32, isOutput=True)

    with ExitStack() as ctx:
        tc = ctx.enter_context(tile.TileContext(nc))
        singles = ctx.enter_context(tc.tile_pool(name="singles", bufs=1))
        adjp = ctx.enter_context(tc.tile_pool(name="adjp", bufs=bufs))
        zp = ctx.enter_context(tc.tile_pool(name="zp", bufs=bufs))
        smalls = ctx.enter_context(tc.tile_pool(name="smalls", bufs=3))
        psp = ctx.enter_context(tc.tile_pool(name="psp", bufs=1, space="PSUM"))
        pst = ctx.enter_context(tc.tile_pool(name="pst", bufs=2, space="PSUM"))

        FB = singles.tile([P, ROWS], f16)
        nc.sync.dma_start(out=FB, in_=fb_d[:, :])
        GS = singles.tile([P, JB], f32)
        nc.sync.dma_start(out=GS, in_=g_d[:, :])
        HA = singles.tile([P, JB, HA_W], f16)
        nc.sync.dma_start(out=HA, in_=ha_d[:, :, :])
        IDT = singles.tile([P, P], f32)
        make_identity(nc, IDT)
        NEGCS = singles.tile([P, 1], f32)
        nc.vector.memset(NEGCS, -CS)

        # persistent psum accumulators, one per 512-wide i-chunk
        ps = [
            psp.tile([HA_W, 512], f32, tag=f"ps{c}", name=f"ps{c}") for c in range(2)
        ]

        rep_ctx = ExitStack()
        if reps > 1:  # timing mode: repeat the whole body inside the NEFF
            rep_ctx.enter_context(tc.For_i(0, reps, 1))

        kb = JB // ns
        for s in range(ns):
            adjf = adjp.tile([P, kb, ROWS], f16, tag="adjf")
            src = adj_d[s * kb * P : (s + 1) * kb * P, :].rearrange(
                "(k p) i -> p k i", p=P
            )
            nc.gpsimd.dma_start(out=adjf, in_=src)

            z = zp.tile([P, kb, ROWS], f16, tag="z")
            # first n_act blocks of the stack do leaky-relu on ACT (Prelu),
            # the remaining (contiguous) blocks on DVE (ts add + fused max)
            n_act = kb if s in act_stacks else act_blocks.get(s, 0)
            for kk in range(n_act):
                b = s * kb + kk
                nc.scalar.activation(
                    out=z[:, kk, :],
                    in_=FB[:, :],
                    func=PRELU,
                    bias=GS[:, b : b + 1],
                    scale=1.0,
                    alpha=0.2,
                )
            if n_act < kb:
                for kk in range(n_act, kb):
                    b = s * kb + kk
                    nc.vector.tensor_scalar(
                        out=z[:, kk, :],
                        in0=FB[:, :],
                        scalar1=GS[:, b : b + 1],
                        scalar2=None,
                        op0=Alu.add,
                    )
                nc.vector.scalar_tensor_tensor(
                    out=z[:, n_act:, :], in0=z[:, n_act:, :], scalar=0.2,
                    in1=z[:, n_act:, :], op0=Alu.mult, op1=Alu.max,
                )
            nc.scalar.activation(
                out=z[:, :, :], in_=z[:, :, :], func=Act.Exp,
                bias=NEGCS[:, :], scale=1.0,
            )
            mask_eng = nc.gpsimd if s in pool_masks else nc.vector
            mask_eng.tensor_tensor(
                out=z[:, :, :], in0=z[:, :, :], in1=adjf[:, :, :], op=Alu.mult
            )

            for kk in range(kb):
                b = s * kb + kk
                for c in range(2):
                    nc.tensor.matmul(
                        ps[c],
                        HA[:, b, :],
                        z[:, kk, c * 512 : (c + 1) * 512],
                        start=(b == 0),
                        stop=(b == JB - 1),
                    )

        # epilogue: transpose psum slices, normalize, leaky_relu(0.01)
        for c in range(2):
            sb1 = smalls.tile([HA_W, 512], f32, tag="sb1")
            nc.vector.tensor_copy(out=sb1, in_=ps[c])
            for q in range(4):
                t = c * 4 + q
                ps2 = pst.tile([P, HA_W], f32, tag="ps2")
                nc.tensor.transpose(
                    ps2, sb1[:, q * P : (q + 1) * P], IDT[:HA_W, :HA_W]
                )
                sb2 = smalls.tile([P, HA_W], f32, tag="sb2")
                nc.vector.tensor_copy(out=sb2, in_=ps2)
                rec = smalls.tile([P, 1], f32, tag="rec")
                nc.vector.reciprocal(rec, sb2[:, F_OUT : F_OUT + 1])
                res = smalls.tile([P, F_OUT], f32, tag="res")
                nc.vector.tensor_scalar(
                    out=res, in0=sb2[:, 0:F_OUT], scalar1=rec, scalar2=None,
                    op0=Alu.mult,
                )
                fin = smalls.tile([P, F_OUT], f32, tag="fin")
                if act_stacks:
                    nc.scalar.activation(
                        out=fin, in_=res, func=PRELU, bias=0.0, scale=1.0,
                        alpha=0.01,
                    )
                else:  # sim path (interp lacks Prelu)
                    nc.vector.scalar_tensor_tensor(
                        out=fin, in0=res, scalar=0.01, in1=res,
                        op0=Alu.mult, op1=Alu.max,
                    )
                nc.sync.dma_start(out=out_d[t * P : (t + 1) * P, :], in_=fin)
        rep_ctx.close()
    nc.finalize()
    return nc


def prep_inputs(input, adj, W, a):
    """Host-side prep: h = input@W, f/g projections, per-core sharding layout."""
    h = np.asarray(input, np.float32) @ np.asarray(W, np.float32)
    av = np.asarray(a, np.float32).reshape(2 * F_OUT)
    f = (h @ av[:F_OUT]).astype(np.float32)
    g = (h @ av[F_OUT:]).astype(np.float32)
    gs = np.ascontiguousarray(g.reshape(JB, P).T)  # gs[p, b] = g[b*128+p]
    ha = np.concatenate([h, np.ones((N, 1), np.float32)], axis=1).astype(np.float16)
    ha_r = np.ascontiguousarray(ha.reshape(JB, P, HA_W).transpose(1, 0, 2))
    adj = np.asarray(adj, np.int32)
    in_maps = []
    for c in range(NCORES):
        fc = f[c * ROWS : (c + 1) * ROWS].astype(np.float16)
        in_maps.append(
            {
                "adjt": np.ascontiguousarray(adj[c * ROWS : (c + 1) * ROWS].T),
                "fbig": np.ascontiguousarray(
                    np.broadcast_to(fc[None, :], (P, ROWS))
                ),
                "gsc": gs,
                "haug": ha_r,
            }
        )
    return in_maps


def kernel(input, adj, W, a, sparse):
    from concourse.bass_utils import run_bass_kernel_spmd

    in_maps = prep_inputs(input, adj, W, a)
    if "nc" not in _nc_cache:
        _nc_cache["nc"] = build_bass()
    nc = _nc_cache["nc"]
    r = run_bass_kernel_spmd(nc, in_maps, list(range(NCORES)))
    out = np.concatenate(
        [np.asarray(r.results[c]["out"]) for c in range(NCORES)], axis=0
    )
    return out[None].astype(np.float32)



# revision 5
# speedup vs baseline: 82.5653x; 82.5653x over previous
"""GAT layer kernel for 8 Trainium2 NeuronCores.

Row-shards the N=8192 destination nodes across 8 cores (1024 rows each).
Each core receives its adjacency slice pre-transposed (adjT[j, i] =
adj[row_i, j], same bytes, sharding layout choice), so all compute runs in
the matmul-ready [j, i] layout and no on-device transpose is needed.

Per core, 8 "stacks", each stack s covering 8 j-blocks of 128 (j in
[s*1024, (s+1)*1024)), as one [128, 8*1024] fp16 tile:
  adjT --dma+cast--> adjf16 (0.0/1.0)                       (SWDGE, int32->fp16)
  per j-block b: z[:,b,:] = leaky_relu(f_i + g_j, 0.2)
      ACT path:  Prelu(in=F, bias=g_b, alpha=0.2)           (1 op/block)
      DVE path:  ts add (f + g_b), then stacked in-place
                 scalar_tensor_tensor max(0.2*z, z)
  z = exp(z - 2)           stacked, ACT (the -2 cancels in softmax; keeps fp16)
  z = z * adjf16           stacked, DVE (mask -> exact zeros)
  per block b, i-chunk c:  psum_c[65, 512] += haug[b].T @ z  (PE; haug=[h|1],
                           row 64 accumulates the softmax denominator)
Epilogue: PE-transpose psum 128-col slices, divide rows by the denominator,
Prelu(0.01), DMA out (natural [1024, 64] layout).

h = input@W, f = h@a[:64], g = h@a[64:] are precomputed on host (the
"replicate h" strategy from the sharding hint - O(N*F) vs O(N^2) on device).
"""

import sys

if "/opt/trn_rl_repo" not in sys.path:
    sys.path.insert(0, "/opt/trn_rl_repo")

import numpy as np

N = 8192
F_OUT = 64
NCORES = 8
ROWS = N // NCORES  # 1024 rows per core
P = 128
JB = N // P         # 64 j-blocks
NS = 8              # stacks per core
KB = JB // NS       # 8 j-blocks per stack
HA_W = F_OUT + 1    # h features + ones column
CS = 2.0            # exp shift (cancels in softmax)
ACT_STACKS = (1, 4)     # stacks whose leaky-relu runs on ACT (Prelu)
POOL_MASKS = ()         # gpsimd mask offload (off: proven regression on HW)
ACT_BLOCKS = {6: 3}     # partial stacks on ACT (balance after u8-adj DMA cut)

_nc_cache = {}


def build_bass(act_stacks=ACT_STACKS, reps=1, sim_relu=False, pool_masks=POOL_MASKS,
               ns=NS, bufs=4, act_blocks=ACT_BLOCKS):
    from contextlib import ExitStack

    import concourse.bacc as bacc
    import concourse.tile as tile
    from concourse import mybir
    from concourse.masks import make_identity

    f16 = mybir.dt.float16
    f32 = mybir.dt.float32
    u8 = mybir.dt.uint8
    Alu = mybir.AluOpType
    Act = mybir.ActivationFunctionType

    # sim_relu: the interp lacks Prelu; Relu has identical cost (timing-only runs)
    PRELU = Act.Relu if sim_relu else Act.Prelu

    nc = bacc.Bacc()
    adj_d = nc.declare_dram_parameter("adjt", [N, ROWS], u8, isOutput=False)
    fb_d = nc.declare_dram_parameter("fbig", [P, ROWS], f16, isOutput=False)
    g_d = nc.declare_dram_parameter("gsc", [P, JB], f32, isOutput=False)
    ha_d = nc.declare_dram_parameter("haug", [P, JB, HA_W], f16, isOutput=False)
    out_d = nc.declare_dram_parameter("out", [ROWS, F_OUT], f32, isOutput=True)

    with ExitStack() as ctx:
        tc = ctx.enter_context(tile.TileContext(nc))
        singles = ctx.enter_context(tc.tile_pool(name="singles", bufs=1))
        adjp = ctx.enter_context(tc.tile_pool(name="adjp", bufs=bufs))
        zp = ctx.enter_context(tc.tile_pool(name="zp", bufs=bufs))
        smalls = ctx.enter_context(tc.tile_pool(name="smalls", bufs=3))
        psp = ctx.enter_context(tc.tile_pool(name="psp", bufs=1, space="PSUM"))
        pst = ctx.enter_context(tc.tile_pool(name="pst", bufs=2, space="PSUM"))

        FB = singles.tile([P, ROWS], f16)
        nc.sync.dma_start(out=FB, in_=fb_d[:, :])
        GS = singles.tile([P, JB], f32)
        nc.sync.dma_start(out=GS, in_=g_d[:, :])
        HA = singles.tile([P, JB, HA_W], f16)
        nc.sync.dma_start(out=HA, in_=ha_d[:, :, :])
        IDT = singles.tile([P, P], f32)
        make_identity(nc, IDT)
        NEGCS = singles.tile([P, 1], f32)
        nc.vector.memset(NEGCS, -CS)

        # persistent psum accumulators, one per 512-wide i-chunk
        ps = [
            psp.tile([HA_W, 512], f32, tag=f"ps{c}", name=f"ps{c}") for c in range(2)
        ]

        rep_ctx = ExitStack()
        if reps > 1:  # timing mode: repeat the whole body inside the NEFF
            rep_ctx.enter_context(tc.For_i(0, reps, 1))

        kb = JB // ns
        for s in range(ns):
            adjf = adjp.tile([P, kb, ROWS], f16, tag="adjf")
            src = adj_d[s * kb * P : (s + 1) * kb * P, :].rearrange(
                "(k p) i -> p k i", p=P
            )
            nc.gpsimd.dma_start(out=adjf, in_=src)

            z = zp.tile([P, kb, ROWS], f16, tag="z")
            # first n_act blocks of the stack do leaky-relu on ACT (Prelu),
            # the remaining (contiguous) blocks on DVE (ts add + fused max)
            n_act = kb if s in act_stacks else act_blocks.get(s, 0)
            for kk in range(n_act):
                b = s * kb + kk
                nc.scalar.activation(
                    out=z[:, kk, :],
                    in_=FB[:, :],
                    func=PRELU,
                    bias=GS[:, b : b + 1],
                    scale=1.0,
                    alpha=0.2,
                )
            if n_act < kb:
                for kk in range(n_act, kb):
                    b = s * kb + kk
                    nc.vector.tensor_scalar(
                        out=z[:, kk, :],
                        in0=FB[:, :],
                        scalar1=GS[:, b : b + 1],
                        scalar2=None,
                        op0=Alu.add,
                    )
                nc.vector.scalar_tensor_tensor(
                    out=z[:, n_act:, :], in0=z[:, n_act:, :], scalar=0.2,
                    in1=z[:, n_act:, :], op0=Alu.mult, op1=Alu.max,
                )
            nc.scalar.activation(
                out=z[:, :, :], in_=z[:, :, :], func=Act.Exp,
                bias=NEGCS[:, :], scale=1.0,
            )
            mask_eng = nc.gpsimd if s in pool_masks else nc.vector
            mask_eng.tensor_tensor(
                out=z[:, :, :], in0=z[:, :, :], in1=adjf[:, :, :], op=Alu.mult
            )

            for kk in range(kb):
                b = s * kb + kk
                for c in range(2):
                    nc.tensor.matmul(
                        ps[c],
                        HA[:, b, :],
                        z[:, kk, c * 512 : (c + 1) * 512],
                        start=(b == 0),
                        stop=(b == JB - 1),
                    )

        # epilogue: transpose psum slices, normalize, leaky_relu(0.01)
        for c in range(2):
            sb1 = smalls.tile([HA_W, 512], f32, tag="sb1")
            nc.vector.tensor_copy(out=sb1, in_=ps[c])
            for q in range(4):
                t = c * 4 + q
                ps2 = pst.tile([P, HA_W], f32, tag="ps2")
                nc.tensor.transpose(
                    ps2, sb1[:, q * P : (q + 1) * P], IDT[:HA_W, :HA_W]
                )
                sb2 = smalls.tile([P, HA_W], f32, tag="sb2")
                nc.vector.tensor_copy(out=sb2, in_=ps2)
                rec = smalls.tile([P, 1], f32, tag="rec")
                nc.vector.reciprocal(rec, sb2[:, F_OUT : F_OUT + 1])
                res = smalls.tile([P, F_OUT], f32, tag="res")
                nc.vector.tensor_scalar(
                    out=res, in0=sb2[:, 0:F_OUT], scalar1=rec, scalar2=None,
                    op0=Alu.mult,
                )
                fin = smalls.tile([P, F_OUT], f32, tag="fin")
                if act_stacks:
                    nc.scalar.activation(
                        out=fin, in_=res, func=PRELU, bias=0.0, scale=1.0,
                        alpha=0.01,
                    )
                else:  # sim path (interp lacks Prelu)
                    nc.vector.scalar_tensor_tensor(
                        out=fin, in0=res, scalar=0.01, in1=res,
                        op0=Alu.mult, op1=Alu.max,
                    )
                nc.sync.dma_start(out=out_d[t * P : (t + 1) * P, :], in_=fin)
        rep_ctx.close()
    nc.finalize()
    return nc


def prep_inputs(input, adj, W, a):
    """Host-side prep: h = input@W, f/g projections, per-core sharding layout."""
    h = np.asarray(input, np.float32) @ np.asarray(W, np.float32)
    av = np.asarray(a, np.float32).reshape(2 * F_OUT)
    f = (h @ av[:F_OUT]).astype(np.float32)
    g = (h @ av[F_OUT:]).astype(np.float32)
    gs = np.ascontiguousarray(g.reshape(JB, P).T)  # gs[p, b] = g[b*128+p]
    ha = np.concatenate([h, np.ones((N, 1), np.float32)], axis=1).astype(np.float16)
    ha_r = np.ascontiguousarray(ha.reshape(JB, P, HA_W).transpose(1, 0, 2))
    adj = np.asarray(adj, np.int32)
    in_maps = []
    for c in range(NCORES):
        fc = f[c * ROWS : (c + 1) * ROWS].astype(np.float16)
        in_maps.append(
            {
                "adjt": np.ascontiguousarray(
                    adj[c * ROWS : (c + 1) * ROWS].T.astype(np.uint8)
                ),
                "fbig": np.ascontiguousarray(
                    np.broadcast_to(fc[None, :], (P, ROWS))
                ),
                "gsc": gs,
                "haug": ha_r,
            }
        )
    return in_maps


def kernel(input, adj, W, a, sparse):
    from concourse.bass_utils import run_bass_kernel_spmd

    in_maps = prep_inputs(input, adj, W, a)
    if "nc" not in _nc_cache:
        _nc_cache["nc"] = build_bass()
    nc = _nc_cache["nc"]
    r = run_bass_kernel_spmd(nc, in_maps, list(range(NCORES)))
    out = np.concatenate(
        [np.asarray(r.results[c]["out"]) for c in range(NCORES)], axis=0
    )
    return out[None].astype(np.float32)



# revision 21
# speedup vs baseline: 87.3082x; 1.0574x over previous
"""GAT layer kernel for 8 Trainium2 NeuronCores.

Row-shards the N=8192 destination nodes across 8 cores (1024 rows each).
Each core gets its adjacency slice as gm[j, i] = g_j + (adj[i_row, j] ? 0
: -300) in fp16, laid out [128, 64, 1024] (partition = j mod 128) so each
stack's DMA is one contiguous 16KB read per partition via HWDGE.

Per core, 8 stacks of 8 j-blocks (one [128, 8, 1024] fp16 tile each):
  t = FB + gm            (DVE tensor_tensor add, fp16 2x; folds f-add AND
                          the adjacency mask: masked entries sit at ~-300)
  leaky_relu(t, 0.2)     (first n_act blocks: ACT Prelu; rest: one DVE
                          stacked scalar_tensor_tensor max(0.2t, t), 2x)
  t = exp(t - 2)         (ACT, stacked; -2 cancels in softmax; masked
                          entries flush to exact fp16 zero)
  per block b, i-chunk c: psum_c[65, 512] += haug[b].T @ t  (PE; haug=[h|1],
                          row 64 accumulates the softmax denominator)
Epilogue: PE-transpose psum 128-col slices, divide rows by the denominator,
Prelu(0.01), DMA out (natural [1024, 64] layout).

h = input@W, f = h@a[:64], g = h@a[64:] are precomputed on host (the
"replicate h" strategy from the sharding hint - O(N*F) vs O(N^2) on device).
"""

import sys

if "/opt/trn_rl_repo" not in sys.path:
    sys.path.insert(0, "/opt/trn_rl_repo")

import numpy as np

N = 8192
F_OUT = 64
NCORES = 8
ROWS = N // NCORES  # 1024 rows per core
P = 128
JB = N // P         # 64 j-blocks
HA_W = F_OUT + 1    # h features + ones column
CS = 2.0            # exp shift (cancels in softmax)
MB = 300.0          # mask offset: exp(0.2*(x-300)-2) == 0 in fp16
NS = 16             # stacks per core (fine stacks pipeline better)
KB = JB // NS
NACT = (2, 2, 2, 1, 2, 2, 2, 1, 2, 2, 2, 1, 2, 2, 2, 1)  # leaky blocks on ACT
NPOOL = (0,) * 16  # leaky blocks on Pool (stt on Pool fails at runtime on HW)

_nc_cache = {}


def build_bass(nact=NACT, npool=NPOOL, reps=1, sim_relu=False, ns=NS, bufs=8):
    from contextlib import ExitStack

    import concourse.bacc as bacc
    import concourse.tile as tile
    from concourse import mybir
    from concourse.masks import make_identity

    f16 = mybir.dt.float16
    f32 = mybir.dt.float32
    Alu = mybir.AluOpType
    Act = mybir.ActivationFunctionType

    # sim_relu: the interp lacks Prelu; Relu has identical cost (timing-only runs)
    PRELU = Act.Relu if sim_relu else Act.Prelu

    nc = bacc.Bacc()
    gm_d = nc.declare_dram_parameter("gmsk", [P, JB, ROWS], f16, isOutput=False)
    fb_d = nc.declare_dram_parameter("fbig", [P, ROWS], f16, isOutput=False)
    ha_d = nc.declare_dram_parameter("haug", [P, JB, HA_W], f16, isOutput=False)
    out_d = nc.declare_dram_parameter("out", [ROWS, F_OUT], f32, isOutput=True)

    with ExitStack() as ctx:
        tc = ctx.enter_context(tile.TileContext(nc))
        singles = ctx.enter_context(tc.tile_pool(name="singles", bufs=1))
        gmp = ctx.enter_context(tc.tile_pool(name="gmp", bufs=bufs))
        smalls = ctx.enter_context(tc.tile_pool(name="smalls", bufs=3))
        psp = ctx.enter_context(tc.tile_pool(name="psp", bufs=1, space="PSUM"))
        pst = ctx.enter_context(tc.tile_pool(name="pst", bufs=2, space="PSUM"))

        FB = singles.tile([P, ROWS], f16)
        nc.sync.dma_start(out=FB, in_=fb_d[:, :])
        HA = singles.tile([P, JB, HA_W], f16)
        nc.sync.dma_start(out=HA, in_=ha_d[:, :, :])
        IDT = singles.tile([P, P], f32)
        make_identity(nc, IDT)
        NEGCS = singles.tile([P, 1], f32)
        nc.vector.memset(NEGCS, -CS)

        # persistent psum accumulators, one per 512-wide i-chunk
        ps = [
            psp.tile([HA_W, 512], f32, tag=f"ps{c}", name=f"ps{c}") for c in range(2)
        ]

        rep_ctx = ExitStack()
        if reps > 1:  # timing mode: repeat the whole body inside the NEFF
            rep_ctx.enter_context(tc.For_i(0, reps, 1))

        kb = JB // ns
        for s in range(ns):
            t = gmp.tile([P, kb, ROWS], f16, tag="gm")
            nc.sync.dma_start(out=t, in_=gm_d[:, s * kb : (s + 1) * kb, :])

            # t = f_i + g_j - 300*(1-adj): per-block 2x tensor_tensor adds
            for kk in range(kb):
                nc.vector.tensor_tensor(
                    out=t[:, kk, :], in0=t[:, kk, :], in1=FB[:, :], op=Alu.add
                )
            # leaky split: [0:nA) ACT Prelu, [nA:nA+nP) Pool stt, rest DVE stt
            n_a = nact[s] if s < len(nact) else 0
            n_p = npool[s] if s < len(npool) else 0
            for kk in range(n_a):
                nc.scalar.activation(
                    out=t[:, kk, :], in_=t[:, kk, :], func=PRELU,
                    bias=0.0, scale=1.0, alpha=0.2,
                )
            if n_p > 0:
                nc.gpsimd.scalar_tensor_tensor(
                    out=t[:, n_a : n_a + n_p, :], in0=t[:, n_a : n_a + n_p, :],
                    scalar=0.2, in1=t[:, n_a : n_a + n_p, :],
                    op0=Alu.mult, op1=Alu.max,
                )
            if n_a + n_p < kb:
                nc.vector.scalar_tensor_tensor(
                    out=t[:, n_a + n_p :, :], in0=t[:, n_a + n_p :, :], scalar=0.2,
                    in1=t[:, n_a + n_p :, :], op0=Alu.mult, op1=Alu.max,
                )
            nc.scalar.activation(
                out=t[:, :, :], in_=t[:, :, :], func=Act.Exp,
                bias=NEGCS[:, :], scale=1.0,
            )

            for kk in range(kb):
                b = s * kb + kk
                for c in range(2):
                    nc.tensor.matmul(
                        ps[c],
                        HA[:, b, :],
                        t[:, kk, c * 512 : (c + 1) * 512],
                        start=(b == 0),
                        stop=(b == JB - 1),
                    )

        # epilogue: transpose psum slices, normalize, leaky_relu(0.01)
        for c in range(2):
            sb1 = smalls.tile([HA_W, 512], f32, tag="sb1")
            nc.vector.tensor_copy(out=sb1, in_=ps[c])
            for q in range(4):
                tq = c * 4 + q
                ps2 = pst.tile([P, HA_W], f32, tag="ps2")
                nc.tensor.transpose(
                    ps2, sb1[:, q * P : (q + 1) * P], IDT[:HA_W, :HA_W]
                )
                sb2 = smalls.tile([P, HA_W], f32, tag="sb2")
                nc.vector.tensor_copy(out=sb2, in_=ps2)
                rec = smalls.tile([P, 1], f32, tag="rec")
                nc.vector.reciprocal(rec, sb2[:, F_OUT : F_OUT + 1])
                res = smalls.tile([P, F_OUT], f32, tag="res")
                nc.vector.tensor_scalar(
                    out=res, in0=sb2[:, 0:F_OUT], scalar1=rec, scalar2=None,
                    op0=Alu.mult,
                )
                fin = smalls.tile([P, F_OUT], f32, tag="fin")
                nc.scalar.activation(
                    out=fin, in_=res, func=PRELU, bias=0.0, scale=1.0,
                    alpha=0.01,
                )
                nc.sync.dma_start(out=out_d[tq * P : (tq + 1) * P, :], in_=fin)
        rep_ctx.close()
    nc.finalize()
    return nc


def prep_inputs(input, adj, W, a):
    """Host-side prep: h = input@W, f/g projections, per-core sharding layout."""
    h = np.asarray(input, np.float32) @ np.asarray(W, np.float32)
    av = np.asarray(a, np.float32).reshape(2 * F_OUT)
    f = (h @ av[:F_OUT]).astype(np.float32)
    g = (h @ av[F_OUT:]).astype(np.float32)
    ha = np.concatenate([h, np.ones((N, 1), np.float32)], axis=1).astype(np.float16)
    ha_r = np.ascontiguousarray(ha.reshape(JB, P, HA_W).transpose(1, 0, 2))
    adj = np.asarray(adj, np.int32)
    in_maps = []
    for c in range(NCORES):
        fc = f[c * ROWS : (c + 1) * ROWS].astype(np.float16)
        adjT = adj[c * ROWS : (c + 1) * ROWS].T  # [j, i]
        gm = (g[:, None] - MB * (adjT == 0)).astype(np.float16)  # [j, i]
        in_maps.append(
            {
                "gmsk": np.ascontiguousarray(
                    gm.reshape(JB, P, ROWS).transpose(1, 0, 2)
                ),
                "fbig": np.ascontiguousarray(
                    np.broadcast_to(fc[None, :], (P, ROWS))
                ),
                "haug": ha_r,
            }
        )
    return in_maps


def kernel(input, adj, W, a, sparse):
    from concourse.bass_utils import run_bass_kernel_spmd

    in_maps = prep_inputs(input, adj, W, a)
    if "nc" not in _nc_cache:
        _nc_cache["nc"] = build_bass()
    nc = _nc_cache["nc"]
    r = run_bass_kernel_spmd(nc, in_maps, list(range(NCORES)))
    out = np.concatenate(
        [np.asarray(r.results[c]["out"]) for c in range(NCORES)], axis=0
    )
    return out[None].astype(np.float32)
